# revision 1
# baseline (speedup 1.0000x reference)
"""2-layer GCN (PyG GCNConv x2 + sigmoid) on 8 TRN2 NeuronCores, single fused NEFF.

Design (memory-regime, gather-bound):
- All GCN normalization is folded out of the per-edge path:
  out = sigmoid(dinv_dst * segsum(M'[src]) + b), with M' = W1^T @ (x * dinv)^T
  built on-device by the PE. Per-edge work is pure gather + sum.
- Layer 1: dst-sharded across cores; feature-major source tables built in 4
  pipelined node-quarters (12.5K nodes each, fits int16 gather indices);
  GPSIMD ap_gather streams per-edge rows (~1.4ns/edge); exact segment sums
  via degree-ladder tensor_reduce with compile-time-uniform budgets across
  cores; perm-gather reassembles node order; finalize is sliced to overlap
  the last quarter's assembly.
- z' = h'@W2 shards are AllGathered on-device (DRAM bounce, Shared output);
  falls back to a two-launch host-crossing variant if collectives fail.
- Layer 2: scalar gathers use the 16-partition-group structure of ap_gather
  (8 independent edge groups per instruction -> 8x throughput); tiny tables
  live on stride-16 partitions only.
- Host does only index/layout preprocessing: degrees, ladder packing with
  degree bucketing, a src-table permutation that dealigns self-loops (keeps
  shared max-over-core budgets ~13% instead of ~34% over actual edges),
  int16 index wrapping (16B-aligned slices), output unpermutation.
"""

import sys

sys.path.insert(0, "/opt/trn_rl_repo")
import numpy as np
from contextlib import ExitStack

from concourse import bacc, mybir
from concourse.tile import TileContext
from concourse.bass_utils import run_bass_kernel_spmd

MEASURE = False  # when True, run the cost-model simulator and fill LAST_SIM_NS
LAST_SIM_NS = None

N = 50000
E = 800000
F = 128
P = 128
NCORES = 8
NSH = N // NCORES  # 6250 dst nodes per core
NQ = 4  # src quarters
QN = N // NQ  # 12500 nodes per quarter
QCOLS = 12800  # quarter table cols: [zero, 12500 nodes, pad] -> 25 chunks of 512
MMCH = 512  # matmul chunk
G1 = 2560  # k1 gather chunk (slots; /16 cols stays 16B-aligned)
NGROUP = 8  # k2: 16-partition groups


def _wrap16(idx_flat):
    """logical sequence -> [16, n/16] wrapped layout"""
    n = idx_flat.shape[0]
    assert n % 16 == 0
    return np.ascontiguousarray(idx_flat.reshape(n // 16, 16).T)


def _pad16(n, mult=16):
    return ((n + mult - 1) // mult) * mult


def _pad128(n):
    return ((n + 127) // 128) * 128


def host_prep(x, edge_index, W1, b1, W2, b2):
    """All index/layout preprocessing. Returns per-core input maps + metadata."""
    src = np.concatenate([edge_index[0], np.arange(N, dtype=np.int64)]).astype(np.int32)
    dst = np.concatenate([edge_index[1], np.arange(N, dtype=np.int64)]).astype(np.int32)
    deg = np.bincount(dst, minlength=N).astype(np.float32)
    dinv = 1.0 / np.sqrt(np.maximum(deg, 1e-12))
    dinv[deg <= 0] = 0.0

    # Permute the src-table node order so each node's self-loop lands in a
    # pseudo-random quarter: keeps per-(core,quarter) degree distributions
    # aligned across cores, which keeps the shared max-over-cores ladder
    # budgets tight. dst-side layout is unaffected.
    psrc = np.random.default_rng(12345).permutation(N)  # node -> table position
    pinv = np.argsort(psrc)  # table position -> node

    # xT_pre[f, pos] = x[node, f] * dinv[node]; layout per quarter: [zero, nodes, pad]
    xtp = (x * dinv[:, None]).T.astype(np.float32)[:, pinv]  # [128, N] position order
    xt = np.zeros((P, NQ * QCOLS), dtype=np.float32)
    for q in range(NQ):
        xt[:, q * QCOLS + 1 : q * QCOLS + 1 + QN] = xtp[:, q * QN : (q + 1) * QN]

    core = dst // NSH  # [Etot]
    dstl = dst % NSH
    pos = psrc[src]
    quarter = pos // QN
    srcl = (pos % QN).astype(np.int32) + 1  # 0 = zero col

    # per (core, quarter): kappa counts per local dst node
    kap = np.zeros((NCORES, NQ, NSH), dtype=np.int32)
    for c in range(NCORES):
        mc = core == c
        for q in range(NQ):
            m = mc & (quarter == q)
            kap[c, q] = np.bincount(dstl[m], minlength=NSH)

    kmax = int(kap.max())
    # bucketed ladder: exact for small degrees, coarse above (pools the sparse
    # tail so the max-over-cores budget inflation stays small)
    lut = np.arange(kmax + 1)
    for kk in range(9, kmax + 1):
        for bb in (10, 12, 14, 17, 21, 26, 32, 40, 48, 64, 96, 128, 192, 256):
            if kk <= bb:
                lut[kk] = bb
                break
    kapb = lut[kap]
    # ladder budgets per quarter: n_b = max over cores of #nodes with bucket==b
    budgets = []  # budgets[q] = {bucket: n_b}
    for q in range(NQ):
        b = {}
        for k in np.unique(kapb[:, q, :]):
            k = int(k)
            if k == 0:
                continue
            nk = int((kapb[:, q, :] == k).sum(axis=1).max())
            if nk > 0:
                b[k] = nk
        budgets.append(b)

    # pack ladder rows into G1-slot chunks; shared layout per quarter
    # descriptors: (chunk_idx, slot_off_in_chunk, n_rows, k, accp_col_off)
    layouts = []  # layouts[q] = (n_chunks, [descr], accp_cols, {k: col_off})
    for q in range(NQ):
        descr = []
        kbase = {}
        col = 1  # col 0 = zero col
        ch, off = 0, 0
        for k in sorted(budgets[q]):
            nk = budgets[q][k]
            kbase[k] = col
            left = nk
            while left > 0:
                fit = min(left, (G1 - off) // k)
                if fit == 0:
                    ch += 1
                    off = 0
                    fit = min(left, G1 // k)
                descr.append((ch, off, fit, k, col))
                off += fit * k
                col += fit
                left -= fit
            # next k continues filling same chunk
        n_chunks = ch + 1
        layouts.append((n_chunks, descr, col, kbase))

    SQ = [layouts[q][0] * G1 for q in range(NQ)]  # slots per quarter
    PQ = max(layouts[q][2] for q in range(NQ))  # accP col budget
    PQ = _pad16(PQ)

    # build per-core slot-index arrays + perms
    eidx = np.zeros((NCORES, sum(SQ)), dtype=np.int16)
    PERM_NI = _pad128(NSH)  # 6272; multiple of 128 so idx slices stay 16B-aligned
    perms = np.zeros((NCORES, NQ, PERM_NI), dtype=np.int16)
    order = np.lexsort((dstl, quarter, core))  # edges grouped by (core, quarter, dst)
    so, do_, qo, co = srcl[order], dstl[order], quarter[order], core[order]
    for c in range(NCORES):
        qbase = 0
        for q in range(NQ):
            m = (co == c) & (qo == q)
            s_cq, d_cq = so[m], do_[m]  # sorted by dst
            kv = kap[c, q]
            kvb = lut[kv]
            # nodes with kappa>0, bucket-grouped: rank within bucket-section
            nodes = np.nonzero(kv)[0]
            kn = kv[nodes]  # actual degree (slots filled)
            knb = kvb[nodes]  # bucket (row width)
            nd_order = np.lexsort((nodes, knb))  # sort nodes by (bucket, node)
            nodes_s = nodes[nd_order]
            kn_s = kn[nd_order]
            knb_s = knb[nd_order]
            # row start slot for each node, following the shared layout
            _, descr, _, kbase = layouts[q]
            # per-bucket: rank of node among same-bucket nodes
            rank = np.zeros(len(nodes_s), dtype=np.int64)
            colof = np.zeros(len(nodes_s), dtype=np.int64)
            for k in np.unique(knb_s):
                mk = knb_s == k
                rank[mk] = np.arange(mk.sum())
                colof[mk] = kbase[int(k)]
            node_col = colof + rank  # accP column of each node
            perms[c, q, : len(nodes)] = 0
            pm = np.zeros(NSH, dtype=np.int16)
            pm[nodes_s] = node_col.astype(np.int16)
            perms[c, q, :NSH] = pm
            # slot position of each (row=node_col, lane): need chunk/slot map per accP col
            col2slot = np.full(layouts[q][2], -1, dtype=np.int64)
            for ch, off, n_rows, k, col in descr:
                cols = np.arange(n_rows)
                col2slot[col + cols] = ch * G1 + off + cols * k
            # edges of node appear consecutively (sorted by dst within (c,q))
            # slot of edge j of node n = col2slot[node_col[n]] + j
            # build via repeat
            starts = col2slot[node_col]
            eslots = np.repeat(starts, kn_s) + _concat_aranges(kn_s)
            # values: srcl of edges, grouped per node ascending-dst...
            # s_cq is sorted by dst; nodes_s is sorted by (k,node) -> reorder edges
            edge_node_ptr = np.zeros(NSH + 1, dtype=np.int64)
            edge_node_ptr[1:] = np.cumsum(kv)
            ev = np.concatenate(
                [s_cq[edge_node_ptr[n] : edge_node_ptr[n + 1]] for n in nodes_s]
            ) if len(nodes_s) else np.zeros(0, dtype=np.int32)
            eidx[c, qbase + eslots] = ev.astype(np.int16)
            qbase += SQ[q]

    # wrap idx arrays
    eidx_w = np.zeros((NCORES, P, sum(SQ) // 16), dtype=np.int16)
    perm_w = np.zeros((NCORES, P, NQ * (PERM_NI // 16)), dtype=np.int16)
    for c in range(NCORES):
        eidx_w[c] = np.tile(_wrap16(eidx[c]), (NGROUP, 1))
        pw = np.concatenate([_wrap16(perms[c, q]) for q in range(NQ)], axis=1)
        perm_w[c] = np.tile(pw, (NGROUP, 1))

    dinvb = np.stack([np.tile(dinv[c * NSH : (c + 1) * NSH], (P, 1)) for c in range(NCORES)])

    meta = dict(layouts=layouts, SQ=SQ, PQ=PQ, PERM_NI=PERM_NI, dinv=dinv)
    k1_inputs = []
    for c in range(NCORES):
        k1_inputs.append(
            {
                "xt": xt,
                "w1": W1.astype(np.float32),
                "b1": b1.astype(np.float32).reshape(P, 1),
                "w2": W2.astype(np.float32),
                "eidx": np.ascontiguousarray(eidx_w[c]),
                "perm": np.ascontiguousarray(perm_w[c]),
                "dinvb": np.ascontiguousarray(dinvb[c].astype(np.float32)),
            }
        )
    return k1_inputs, meta, (src, dst, dinv)


def _concat_aranges(lens):
    """[2,3] -> [0,1,0,1,2]"""
    if len(lens) == 0:
        return np.zeros(0, dtype=np.int64)
    total = int(lens.sum())
    out = np.ones(total, dtype=np.int64)
    ends = np.cumsum(lens)
    out[0] = 0
    out[ends[:-1]] = -(lens[:-1] - 1)
    return np.cumsum(out)


def build_k1(meta, debug_acc=False):
    layouts, SQ, PQ, PERM_NI = meta["layouts"], meta["SQ"], meta["PQ"], meta["PERM_NI"]
    nc = bacc.Bacc(None, target_bir_lowering=False)
    f32, i16 = mybir.dt.float32, mybir.dt.int16
    xt_d = nc.dram_tensor("xt", [P, NQ * QCOLS], f32, kind="ExternalInput")
    w1_d = nc.dram_tensor("w1", [P, P], f32, kind="ExternalInput")
    b1_d = nc.dram_tensor("b1", [P, 1], f32, kind="ExternalInput")
    w2_d = nc.dram_tensor("w2", [P, 1], f32, kind="ExternalInput")
    eidx_d = nc.dram_tensor("eidx", [P, sum(SQ) // 16], i16, kind="ExternalInput")
    perm_d = nc.dram_tensor("perm", [P, NQ * (PERM_NI // 16)], i16, kind="ExternalInput")
    dinvb_d = nc.dram_tensor("dinvb", [P, NSH], f32, kind="ExternalInput")
    zout_d = nc.dram_tensor("zout", [1, NSH], f32, kind="ExternalOutput")
    accout_d = (
        nc.dram_tensor("accout", [P, NSH], f32, kind="ExternalOutput") if debug_acc else None
    )

    with ExitStack() as ctx:
        tc = ctx.enter_context(TileContext(nc))
        cpool = ctx.enter_context(tc.tile_pool(name="cpool", bufs=1))
        apool = ctx.enter_context(tc.tile_pool(name="apool", bufs=1))
        w1 = cpool.tile([P, P], f32)
        b1 = cpool.tile([P, 1], f32)
        w2 = cpool.tile([P, 1], f32)
        eidx = cpool.tile([P, sum(SQ) // 16], i16)
        perm = cpool.tile([P, NQ * (PERM_NI // 16)], i16)
        acc = apool.tile([P, NSH], f32)
        accp = apool.tile([P, PQ], f32)
        nc.sync.dma_start(out=w1[:], in_=w1_d[:])
        nc.sync.dma_start(out=b1[:], in_=b1_d[:])
        nc.sync.dma_start(out=w2[:], in_=w2_d[:])
        nc.sync.dma_start(out=eidx[:], in_=eidx_d[:])
        nc.sync.dma_start(out=perm[:], in_=perm_d[:])
        nc.vector.memset(accp[:, 0:1], 0.0)

        with (
            tc.tile_pool(name="tabs", bufs=2) as tabs,
            tc.tile_pool(name="xpool", bufs=3) as xpool,
            tc.tile_pool(name="gpool", bufs=2) as gpool,
            tc.tile_pool(name="pspool", bufs=2, space="PSUM") as pspool,
        ):
            sq_base = 0
            for q in range(NQ):
                n_chunks, descr, _, _ = layouts[q]
                tab = tabs.tile([P, QCOLS], f32, tag="tab")
                # build quarter table: tab = W1^T @ xt[:, quarter]
                XB = 2 * MMCH  # 1024-col x loads (524KB DMAs)
                for x0 in range(0, QCOLS, XB):
                    xw = min(XB, QCOLS - x0)
                    xc = xpool.tile([P, XB], f32, tag="x")
                    nc.sync.dma_start(
                        out=xc[:, :xw], in_=xt_d[:, q * QCOLS + x0 : q * QCOLS + x0 + xw]
                    )
                    for m0 in range(0, xw, MMCH):
                        ps = pspool.tile([P, MMCH], f32, tag="ps")
                        nc.tensor.matmul(ps[:], w1[:], xc[:, m0 : m0 + MMCH], start=True, stop=True)
                        nc.scalar.activation(
                            tab[:, x0 + m0 : x0 + m0 + MMCH], ps[:],
                            mybir.ActivationFunctionType.Copy,
                        )
                # gather + ladder reduces
                by_chunk = {}
                for d in descr:
                    by_chunk.setdefault(d[0], []).append(d)
                for ch in range(n_chunks):
                    g = gpool.tile([P, G1], f32, tag="g")
                    i0 = (sq_base + ch * G1) // 16
                    nc.gpsimd.ap_gather(
                        g[:], tab[:], eidx[:, i0 : i0 + G1 // 16],
                        channels=P, num_elems=QCOLS, d=1, num_idxs=G1,
                    )
                    for (_, off, n_rows, k, col) in by_chunk.get(ch, []):
                        nc.vector.tensor_reduce(
                            accp[:, col : col + n_rows],
                            g[:, off : off + n_rows * k].rearrange(
                                "p (a b) -> p a b", a=n_rows, b=k
                            ),
                            axis=mybir.AxisListType.X, op=mybir.AluOpType.add,
                        )
                # assemble: acc (+)= accp[perm] in G1-col pieces
                pbase = q * (PERM_NI // 16)
                for s0 in range(0, PERM_NI, G1):
                    w = min(G1, PERM_NI - s0)
                    w = min(w, NSH - s0) if s0 < NSH else 0
                    if w <= 0:
                        break
                    wp = _pad16(w)
                    t = gpool.tile([P, G1], f32, tag="g")
                    nc.gpsimd.ap_gather(
                        t[:, :wp], accp[:], perm[:, pbase + s0 // 16 : pbase + (s0 + wp) // 16],
                        channels=P, num_elems=PQ, d=1, num_idxs=wp,
                    )
                    if q == 0:
                        nc.scalar.activation(
                            acc[:, s0 : s0 + w], t[:, :w], mybir.ActivationFunctionType.Copy
                        )
                    else:
                        nc.vector.tensor_add(acc[:, s0 : s0 + w], acc[:, s0 : s0 + w], t[:, :w])
                sq_base += SQ[q]

        if debug_acc:
            nc.sync.dma_start(out=accout_d[:], in_=acc[:])
        # finalize: h' = dinv*sigmoid(dinv*acc + b1); z' = W2^T @ h'
        with (
            tc.tile_pool(name="fin", bufs=1) as fin,
            tc.tile_pool(name="zpspool", bufs=2, space="PSUM") as zps,
        ):
            dinvb = fin.tile([P, NSH], f32)
            zrow = fin.tile([1, NSH], f32)
            nc.sync.dma_start(out=dinvb[:], in_=dinvb_d[:])
            nc.vector.tensor_mul(acc[:], acc[:], dinvb[:])
            nc.scalar.activation(acc[:], acc[:], mybir.ActivationFunctionType.Sigmoid, bias=b1[:, 0:1])
            nc.vector.tensor_mul(acc[:], acc[:], dinvb[:])
            for m0 in range(0, NSH, MMCH):
                w = min(MMCH, NSH - m0)
                ps = zps.tile([1, MMCH], f32, tag="zps")
                nc.tensor.matmul(ps[:, :w], w2[:], acc[:, m0 : m0 + w], start=True, stop=True)
                nc.scalar.activation(zrow[:, m0 : m0 + w], ps[:, :w], mybir.ActivationFunctionType.Copy)
            nc.sync.dma_start(out=zout_d[:], in_=zrow[:])
    nc.finalize()
    return nc


def host_prep_k2(zfull, src, dst, dinv, b2):
    """Layer-2: scalar gather with 8 independent 16-partition groups."""
    core = dst // NSH
    dstl = dst % NSH
    quarter = src // QN
    srcl = (src % QN).astype(np.int32) + 1
    grp = dstl % NGROUP  # node -> group

    # kappa per (core, quarter, group, node-within-group)
    GN = NSH // NGROUP  # 781.25 -> careful: use dstl//NGROUP as local id (0..781)
    gid = dstl // NGROUP
    GNN = (NSH + NGROUP - 1) // NGROUP  # 782
    kap = np.zeros((NCORES, NQ, NGROUP, GNN), dtype=np.int32)
    for c in range(NCORES):
        mc = core == c
        for q in range(NQ):
            mq = mc & (quarter == q)
            for g in range(NGROUP):
                m = mq & (grp == g)
                kap[c, q, g] = np.bincount(gid[m], minlength=GNN)

    kmax = int(kap.max())
    lut = np.arange(kmax + 1)
    for kk in range(5, kmax + 1):
        for bb in (6, 8, 10, 12, 15, 19, 24, 30, 38, 48, 64, 96, 128, 192, 256):
            if kk <= bb:
                lut[kk] = bb
                break
    kapb = lut[kap]
    budgets, layouts = [], []
    for q in range(NQ):
        b = {}
        for k in np.unique(kapb[:, q, :, :]):
            k = int(k)
            if k == 0:
                continue
            nk = int((kapb[:, q, :, :] == k).sum(axis=2).max())
            if nk > 0:
                b[k] = nk
        budgets.append(b)
        descr, kbase = [], {}
        col = 1
        slots = 0
        for k in sorted(b):
            kbase[k] = col
            descr.append((slots, b[k], k, col))
            slots += b[k] * k
            col += b[k]
        slots = _pad128(slots)
        layouts.append((slots, descr, col, kbase))

    P2 = _pad128(max(l[2] for l in layouts) if layouts else 128)
    SQ2 = [l[0] for l in layouts]

    # z tables: [8, QCOLS2] per quarter, col0=0
    QC2 = QN + 1
    ztab = None
    if zfull is not None:
        ztab = np.zeros((NQ, NGROUP, QC2), dtype=np.float32)
        for q in range(NQ):
            ztab[q, :, 1:] = zfull[q * QN : (q + 1) * QN][None, :]

    eidx2 = np.zeros((NCORES, NGROUP, sum(SQ2)), dtype=np.int16)
    perm2 = np.zeros((NCORES, NGROUP, P2), dtype=np.int16)
    nodemap = np.full((NCORES, NGROUP, P2), -1, dtype=np.int64)  # -> global node
    order = np.lexsort((gid, grp, quarter, core))
    so, go_, qo, co, gi = srcl[order], grp[order], quarter[order], core[order], gid[order]
    for c in range(NCORES):
        for g in range(NGROUP):
            qbase = 0
            for q in range(NQ):
                m = (co == c) & (go_ == g) & (qo == q)
                s_e, gi_e = so[m], gi[m]
                kv = kap[c, q, g]
                kvb = lut[kv]
                nodes = np.nonzero(kv)[0]
                kn = kv[nodes]
                knb = kvb[nodes]
                nd = np.lexsort((nodes, knb))
                nodes_s, kn_s, knb_s = nodes[nd], kn[nd], knb[nd]
                _, descr, _, kbase = layouts[q]
                rank = np.zeros(len(nodes_s), dtype=np.int64)
                colof = np.zeros(len(nodes_s), dtype=np.int64)
                for k in np.unique(knb_s):
                    mk = knb_s == k
                    rank[mk] = np.arange(mk.sum())
                    colof[mk] = kbase[int(k)]
                node_col = colof + rank
                col2slot = np.full(layouts[q][2], -1, dtype=np.int64)
                for soff, n_rows, k, col in descr:
                    cols = np.arange(n_rows)
                    col2slot[col + cols] = soff + cols * k
                starts = col2slot[node_col]
                eslots = np.repeat(starts, kn_s) + _concat_aranges(kn_s)
                ptr = np.zeros(GNN + 1, dtype=np.int64)
                ptr[1:] = np.cumsum(kv)
                ev = (
                    np.concatenate([s_e[ptr[n] : ptr[n + 1]] for n in nodes_s])
                    if len(nodes_s)
                    else np.zeros(0, dtype=np.int32)
                )
                eidx2[c, g, qbase + eslots] = ev.astype(np.int16)
                qbase += SQ2[q]
                # perm for this quarter accumulates into same node cols later;
                # here: node n (local gid) col in accp_q
                # we need per-quarter perms; store packed later
            # perms built per quarter below

    # per-quarter perms + final node mapping
    perm2q = np.zeros((NCORES, NGROUP, NQ, P2), dtype=np.int16)
    for c in range(NCORES):
        for g in range(NGROUP):
            for q in range(NQ):
                kv = kap[c, q, g]
                kvb = lut[kv]
                nodes = np.nonzero(kv)[0]
                knb = kvb[nodes]
                nd = np.lexsort((nodes, knb))
                nodes_s, knb_s = nodes[nd], knb[nd]
                _, _, _, kbase = layouts[q]
                rank = np.zeros(len(nodes_s), dtype=np.int64)
                colof = np.zeros(len(nodes_s), dtype=np.int64)
                for k in np.unique(knb_s):
                    mk = knb_s == k
                    rank[mk] = np.arange(mk.sum())
                    colof[mk] = kbase[int(k)]
                pm = np.zeros(GNN, dtype=np.int16)
                pm[nodes_s] = (colof + rank).astype(np.int16)
                perm2q[c, g, q, :GNN] = pm
            for j in range(GNN):
                n_global = (c * NSH) + (j * NGROUP + g)
                if j * NGROUP + g < NSH:
                    nodemap[c, g, j] = n_global

    # wrapped arrays
    eidx2_w = np.zeros((NCORES, P, sum(SQ2) // 16), dtype=np.int16)
    perm2_w = np.zeros((NCORES, P, NQ * (P2 // 16)), dtype=np.int16)
    for c in range(NCORES):
        for g in range(NGROUP):
            eidx2_w[c, g * 16 : (g + 1) * 16] = _wrap16(eidx2[c, g])
            perm2_w[c, g * 16 : (g + 1) * 16] = np.concatenate(
                [_wrap16(perm2q[c, g, q]) for q in range(NQ)], axis=1
            )

    dinvP = np.zeros((NCORES, NGROUP, P2), dtype=np.float32)
    for c in range(NCORES):
        for g in range(NGROUP):
            for j in range(GNN):
                n = j * NGROUP + g
                if n < NSH:
                    dinvP[c, g, j] = dinv[c * NSH + n]

    meta2 = dict(layouts=layouts, SQ2=SQ2, P2=P2, QC2=QC2, nodemap=nodemap, b2=float(b2[0]))
    k2_inputs = []
    for c in range(NCORES):
        d = {
            "eidx2": np.ascontiguousarray(eidx2_w[c]),
            "perm2": np.ascontiguousarray(perm2_w[c]),
            "dinvp": np.ascontiguousarray(dinvP[c]),
        }
        if ztab is not None:
            d["ztab"] = np.ascontiguousarray(ztab.reshape(NQ * NGROUP, QC2))
        k2_inputs.append(d)
    return k2_inputs, meta2


def build_k2(meta2):
    layouts, SQ2, P2, QC2 = meta2["layouts"], meta2["SQ2"], meta2["P2"], meta2["QC2"]
    b2 = meta2["b2"]
    nc = bacc.Bacc(None, target_bir_lowering=False)
    f32, i16 = mybir.dt.float32, mybir.dt.int16
    ztab_d = nc.dram_tensor("ztab", [NQ * NGROUP, QC2], f32, kind="ExternalInput")
    eidx_d = nc.dram_tensor("eidx2", [P, sum(SQ2) // 16], i16, kind="ExternalInput")
    perm_d = nc.dram_tensor("perm2", [P, NQ * (P2 // 16)], i16, kind="ExternalInput")
    dinvp_d = nc.dram_tensor("dinvp", [NGROUP, P2], f32, kind="ExternalInput")
    out_d = nc.dram_tensor("out2", [NGROUP, P2], f32, kind="ExternalOutput")

    with ExitStack() as ctx:
        tc = ctx.enter_context(TileContext(nc))
        pool = ctx.enter_context(tc.tile_pool(name="pool", bufs=1))
        gpool = ctx.enter_context(tc.tile_pool(name="g2", bufs=3))
        eidx = pool.tile([P, sum(SQ2) // 16], i16)
        perm = pool.tile([P, NQ * (P2 // 16)], i16)
        acc = pool.tile([P, P2], f32)
        accp = pool.tile([P, P2], f32)
        dinvp = pool.tile([P, P2], f32)
        nc.sync.dma_start(out=eidx[:], in_=eidx_d[:])
        nc.sync.dma_start(out=perm[:], in_=perm_d[:])
        nc.sync.dma_start(out=dinvp[0:NGROUP * 16:16, :], in_=dinvp_d[:])
        nc.vector.memset(accp[:, 0:1], 0.0)

        with tc.tile_pool(name="ztabs", bufs=2) as ztabs:
            sq_base = 0
            for q in range(NQ):
                slots, descr, _, _ = layouts[q]
                zt = ztabs.tile([P, QC2], f32, tag="zt")
                nc.sync.dma_start(
                    out=zt[0:NGROUP * 16:16, :], in_=ztab_d[q * NGROUP : (q + 1) * NGROUP, :]
                )
                g = gpool.tile([P, max(_pad16(max(SQ2)), 16)], f32, tag="g")
                nc.gpsimd.ap_gather(
                    g[:, :slots], zt[:], eidx[:, sq_base // 16 : (sq_base + slots) // 16],
                    channels=P, num_elems=QC2, d=1, num_idxs=slots,
                )
                for soff, n_rows, k, col in descr:
                    nc.vector.tensor_reduce(
                        accp[:, col : col + n_rows],
                        g[:, soff : soff + n_rows * k].rearrange("p (a b) -> p a b", a=n_rows, b=k),
                        axis=mybir.AxisListType.X, op=mybir.AluOpType.add,
                    )
                t = gpool.tile([P, max(_pad16(max(SQ2)), 16)], f32, tag="g")
                nc.gpsimd.ap_gather(
                    t[:, :P2], accp[:], perm[:, q * (P2 // 16) : (q + 1) * (P2 // 16)],
                    channels=P, num_elems=P2, d=1, num_idxs=P2,
                )
                if q == 0:
                    nc.vector.tensor_copy(acc[:], t[:, :P2])
                else:
                    nc.vector.tensor_add(acc[:], acc[:], t[:, :P2])
                sq_base += slots

        nc.vector.tensor_mul(acc[:], acc[:], dinvp[:])
        nc.scalar.activation(acc[:], acc[:], mybir.ActivationFunctionType.Sigmoid, bias=b2)
        nc.sync.dma_start(out=out_d[:], in_=acc[0:NGROUP * 16:16, :])
    nc.finalize()
    return nc


def _sim_ns(nc):
    from concourse import bass_interp

    sim = bass_interp.CoreSim(nc, no_exec=True, publish_trace=False)
    sim.simulate()
    return int(sim.time)


def build_fused(meta, meta2):
    """Single-launch: layer 1 + on-device AllGather of z' + layer 2."""
    layouts, SQ, PQ, PERM_NI = meta["layouts"], meta["SQ"], meta["PQ"], meta["PERM_NI"]
    layouts2, SQ2, P2, QC2 = meta2["layouts"], meta2["SQ2"], meta2["P2"], meta2["QC2"]
    b2 = meta2["b2"]
    nc = bacc.Bacc(None, target_bir_lowering=False)
    f32, i16 = mybir.dt.float32, mybir.dt.int16
    xt_d = nc.dram_tensor("xt", [P, NQ * QCOLS], f32, kind="ExternalInput")
    w1_d = nc.dram_tensor("w1", [P, P], f32, kind="ExternalInput")
    b1_d = nc.dram_tensor("b1", [P, 1], f32, kind="ExternalInput")
    w2_d = nc.dram_tensor("w2", [P, 1], f32, kind="ExternalInput")
    eidx_d = nc.dram_tensor("eidx", [P, sum(SQ) // 16], i16, kind="ExternalInput")
    perm_d = nc.dram_tensor("perm", [P, NQ * (PERM_NI // 16)], i16, kind="ExternalInput")
    dinvb_d = nc.dram_tensor("dinvb", [P, NSH], f32, kind="ExternalInput")
    eidx2_d = nc.dram_tensor("eidx2", [P, sum(SQ2) // 16], i16, kind="ExternalInput")
    perm2_d = nc.dram_tensor("perm2", [P, NQ * (P2 // 16)], i16, kind="ExternalInput")
    dinvp_d = nc.dram_tensor("dinvp", [NGROUP, P2], f32, kind="ExternalInput")
    out_d = nc.dram_tensor("out2", [NGROUP, P2], f32, kind="ExternalOutput")

    with ExitStack() as ctx:
        tc = ctx.enter_context(TileContext(nc))
        cpool = ctx.enter_context(tc.tile_pool(name="cpool", bufs=1))
        dram = ctx.enter_context(tc.tile_pool(name="dram", bufs=1, space="DRAM"))
        w1 = cpool.tile([P, P], f32)
        b1 = cpool.tile([P, 1], f32)
        w2 = cpool.tile([P, 1], f32)
        eidx = cpool.tile([P, sum(SQ) // 16], i16)
        perm = cpool.tile([P, NQ * (PERM_NI // 16)], i16)
        zin = nc.dram_tensor("zin_cc", [NGROUP, NSH], f32, kind="Internal")
        zall = nc.dram_tensor("zall_cc", [NGROUP * NCORES, NSH], f32, kind="Internal", addr_space="Shared")
        nc.sync.dma_start(out=w1[:], in_=w1_d[:])
        nc.sync.dma_start(out=b1[:], in_=b1_d[:])
        nc.sync.dma_start(out=w2[:], in_=w2_d[:])
        nc.sync.dma_start(out=eidx[:], in_=eidx_d[:])
        nc.sync.dma_start(out=perm[:], in_=perm_d[:])

        with tc.tile_pool(name="apool", bufs=1) as apool:
            acc = apool.tile([P, NSH], f32)
            accp = apool.tile([P, PQ], f32)
            nc.vector.memset(accp[:, 0:1], 0.0)
            with (
                tc.tile_pool(name="tabs", bufs=2) as tabs,
                tc.tile_pool(name="xpool", bufs=2) as xpool,
                tc.tile_pool(name="gpool", bufs=2) as gpool,
                tc.tile_pool(name="pspool", bufs=2, space="PSUM") as pspool,
            ):
                sq_base = 0
                for q in range(NQ):
                    n_chunks, descr, _, _ = layouts[q]
                    tab = tabs.tile([P, QCOLS], f32, tag="tab")
                    XB = 2 * MMCH
                    for x0 in range(0, QCOLS, XB):
                        xw = min(XB, QCOLS - x0)
                        xc = xpool.tile([P, XB], f32, tag="x")
                        nc.sync.dma_start(
                            out=xc[:, :xw], in_=xt_d[:, q * QCOLS + x0 : q * QCOLS + x0 + xw]
                        )
                        for m0 in range(0, xw, MMCH):
                            ps = pspool.tile([P, MMCH], f32, tag="ps")
                            nc.tensor.matmul(ps[:], w1[:], xc[:, m0 : m0 + MMCH], start=True, stop=True)
                            nc.scalar.activation(
                                tab[:, x0 + m0 : x0 + m0 + MMCH], ps[:],
                                mybir.ActivationFunctionType.Copy,
                            )
                    by_chunk = {}
                    for d_ in descr:
                        by_chunk.setdefault(d_[0], []).append(d_)
                    for ch in range(n_chunks):
                        g = gpool.tile([P, G1], f32, tag="g")
                        i0 = (sq_base + ch * G1) // 16
                        nc.gpsimd.ap_gather(
                            g[:], tab[:], eidx[:, i0 : i0 + G1 // 16],
                            channels=P, num_elems=QCOLS, d=1, num_idxs=G1,
                        )
                        for (_, off, n_rows, k, col) in by_chunk.get(ch, []):
                            nc.vector.tensor_reduce(
                                accp[:, col : col + n_rows],
                                g[:, off : off + n_rows * k].rearrange(
                                    "p (a b) -> p a b", a=n_rows, b=k
                                ),
                                axis=mybir.AxisListType.X, op=mybir.AluOpType.add,
                            )
                    pbase = q * (PERM_NI // 16)
                    for s0 in range(0, PERM_NI, G1):
                        w = min(G1, PERM_NI - s0)
                        w = min(w, NSH - s0) if s0 < NSH else 0
                        if w <= 0:
                            break
                        wp = _pad16(w)
                        t = gpool.tile([P, G1], f32, tag="g")
                        nc.gpsimd.ap_gather(
                            t[:, :wp], accp[:], perm[:, pbase + s0 // 16 : pbase + (s0 + wp) // 16],
                            channels=P, num_elems=PQ, d=1, num_idxs=wp,
                        )
                        if q == 0:
                            nc.scalar.activation(
                                acc[:, s0 : s0 + w], t[:, :w], mybir.ActivationFunctionType.Copy
                            )
                        else:
                            nc.vector.tensor_add(acc[:, s0 : s0 + w], acc[:, s0 : s0 + w], t[:, :w])
                    sq_base += SQ[q]

            with (
                tc.tile_pool(name="fin", bufs=1) as fin,
                tc.tile_pool(name="zpspool", bufs=2, space="PSUM") as zps,
            ):
                dinvb = fin.tile([P, NSH], f32)
                zrow = fin.tile([1, NSH], f32)
                nc.sync.dma_start(out=dinvb[:], in_=dinvb_d[:])
                for f0 in range(0, NSH, G1):
                    fw = min(G1, NSH - f0)
                    sl = slice(f0, f0 + fw)
                    nc.vector.tensor_mul(acc[:, sl], acc[:, sl], dinvb[:, sl])
                    nc.scalar.activation(
                        acc[:, sl], acc[:, sl], mybir.ActivationFunctionType.Sigmoid, bias=b1[:, 0:1]
                    )
                    nc.vector.tensor_mul(acc[:, sl], acc[:, sl], dinvb[:, sl])
                for m0 in range(0, NSH, MMCH):
                    w = min(MMCH, NSH - m0)
                    ps = zps.tile([1, MMCH], f32, tag="zps")
                    nc.tensor.matmul(ps[:, :w], w2[:], acc[:, m0 : m0 + w], start=True, stop=True)
                    nc.scalar.activation(zrow[:, m0 : m0 + w], ps[:, :w], mybir.ActivationFunctionType.Copy)
                for g_ in range(NGROUP):
                    nc.sync.dma_start(out=zin[g_ : g_ + 1, :], in_=zrow[:])

        nc.gpsimd.collective_compute(
            "AllGather", mybir.AluOpType.bypass,
            replica_groups=[list(range(NCORES))],
            ins=[zin[:].opt()], outs=[zall[:].opt()],
        )

        # ---- layer 2 ----
        with (
            tc.tile_pool(name="k2pool", bufs=1) as pool2,
            tc.tile_pool(name="ztabs", bufs=2) as ztabs,
            tc.tile_pool(name="g2", bufs=3) as gpool2,
        ):
            eidx2 = pool2.tile([P, sum(SQ2) // 16], i16)
            perm2 = pool2.tile([P, NQ * (P2 // 16)], i16)
            acc2 = pool2.tile([P, P2], f32)
            accp2 = pool2.tile([P, P2], f32)
            dinvp = pool2.tile([P, P2], f32)
            nc.sync.dma_start(out=eidx2[:], in_=eidx2_d[:])
            nc.sync.dma_start(out=perm2[:], in_=perm2_d[:])
            nc.sync.dma_start(out=dinvp[0 : NGROUP * 16 : 16, :], in_=dinvp_d[:])
            nc.vector.memset(accp2[:, 0:1], 0.0)
            GSZ = max(_pad16(max(SQ2)), P2, 16)
            half = NSH
            sq_base = 0
            for q in range(NQ):
                slots, descr, _, _ = layouts2[q]
                zt = ztabs.tile([P, QC2], f32, tag="zt")
                nc.vector.memset(zt[:, 0:1], 0.0)
                nc.sync.dma_start(
                    out=zt[0 : NGROUP * 16 : 16, 1 : 1 + half],
                    in_=zall[NGROUP * (2 * q) : NGROUP * (2 * q) + NGROUP, :],
                )
                nc.sync.dma_start(
                    out=zt[0 : NGROUP * 16 : 16, 1 + half : 1 + 2 * half],
                    in_=zall[NGROUP * (2 * q + 1) : NGROUP * (2 * q + 1) + NGROUP, :],
                )
                g = gpool2.tile([P, GSZ], f32, tag="g")
                nc.gpsimd.ap_gather(
                    g[:, :slots], zt[:], eidx2[:, sq_base // 16 : (sq_base + slots) // 16],
                    channels=P, num_elems=QC2, d=1, num_idxs=slots,
                )
                for soff, n_rows, k, col in descr:
                    nc.vector.tensor_reduce(
                        accp2[:, col : col + n_rows],
                        g[:, soff : soff + n_rows * k].rearrange("p (a b) -> p a b", a=n_rows, b=k),
                        axis=mybir.AxisListType.X, op=mybir.AluOpType.add,
                    )
                t = gpool2.tile([P, GSZ], f32, tag="g")
                nc.gpsimd.ap_gather(
                    t[:, :P2], accp2[:], perm2[:, q * (P2 // 16) : (q + 1) * (P2 // 16)],
                    channels=P, num_elems=P2, d=1, num_idxs=P2,
                )
                if q == 0:
                    nc.scalar.activation(acc2[:], t[:, :P2], mybir.ActivationFunctionType.Copy)
                else:
                    nc.vector.tensor_add(acc2[:], acc2[:], t[:, :P2])
                sq_base += slots

            nc.vector.tensor_mul(acc2[:], acc2[:], dinvp[:])
            nc.scalar.activation(acc2[:], acc2[:], mybir.ActivationFunctionType.Sigmoid, bias=b2)
            nc.sync.dma_start(out=out_d[:], in_=acc2[0 : NGROUP * 16 : 16, :])
    nc.finalize()
    return nc


def _assemble_out(results, meta2):
    out = np.zeros((N, 1), dtype=np.float32)
    nodemap = meta2["nodemap"]
    for c in range(NCORES):
        o = results[c]["out2"]  # [8, P2]
        valid = nodemap[c] >= 0
        out[nodemap[c][valid], 0] = o[valid]
    return out


def kernel(x, edge_index, W1, b1, W2, b2):
    global LAST_SIM_NS
    x = np.asarray(x, dtype=np.float32)
    edge_index = np.asarray(edge_index)
    k1_inputs, meta, (src, dst, dinv) = host_prep(x, edge_index, W1, b1, W2, b2)
    b2np = np.asarray(b2, dtype=np.float32)
    try:
        # single launch: layer1 + AllGather(z') + layer2 fused in one NEFF
        k2_inputs, meta2 = host_prep_k2(None, src, dst, dinv, b2np)
        nc = build_fused(meta, meta2)
        if MEASURE:
            LAST_SIM_NS = _sim_ns(nc)
        in_maps = [dict(k1_inputs[c], **k2_inputs[c]) for c in range(NCORES)]
        res = run_bass_kernel_spmd(nc, in_maps, list(range(NCORES)))
        return _assemble_out(res.results, meta2)
    except Exception:
        import traceback

        traceback.print_exc()

    # fallback: two launches with z' crossing via host
    nc1 = build_k1(meta)
    sim1 = _sim_ns(nc1) if MEASURE else 0
    res1 = run_bass_kernel_spmd(nc1, k1_inputs, list(range(NCORES)))
    zfull = np.concatenate([res1.results[c]["zout"][0, :NSH] for c in range(NCORES)])
    k2_inputs, meta2 = host_prep_k2(zfull, src, dst, dinv, b2np)
    nc2 = build_k2(meta2)
    if MEASURE:
        LAST_SIM_NS = sim1 + _sim_ns(nc2)
    res2 = run_bass_kernel_spmd(nc2, k2_inputs, list(range(NCORES)))
    return _assemble_out(res2.results, meta2)



# revision 4
# speedup vs baseline: 2.8692x; 2.8692x over previous
"""2-layer GCN on 8 TRN2 cores — v2 single fused NEFF.

- Aggregation commutes with the dense matmuls (GCN is linear before the
  sigmoid), so layer 1 gathers raw x~ = x*dinv columns and applies W1 once
  per core on the 6250-node accumulator (13 matmuls) instead of building
  W1^T@x tables per quarter (eliminates ~100 matmuls + PSUM-drain copies).
- Layer 1: dst-sharded; src space split in 8 quarters of 6250 nodes
  (tables [128, 6400] f32 DMA'd straight from DRAM). Per quarter: 2
  ap_gathers with num_idxs >= table width (the cost model charges
  max(num_idxs, num_elems)), degree-ladder tensor_reduce into accp,
  perm-gather to node order, merged across quarters via identity-matmul
  PSUM accumulation (head 4096 cols) + DVE adds (tail 2176 cols).
- z' = dinv * W2^T h' AllGathered as [8, 6250] (no 8x group replication).
- Layer 2: 8 gather groups = src shards on 16-partition stripes;
  self-loops excluded from the gather (they concentrate in the diagonal
  (c,g) cell and inflate the shared slot budget 55%) and instead injected
  via a spare partition row summed by the ones-matmul partition reduction.
"""

import sys

sys.path.insert(0, "/opt/trn_rl_repo")
import numpy as np
import ml_dtypes
from contextlib import ExitStack

from concourse import bacc, mybir
from concourse.tile import TileContext
from concourse.bass_utils import run_bass_kernel_spmd

MEASURE = False  # when True, run the cost-model simulator and fill LAST_SIM_NS
LAST_SIM_NS = None

N = 50000
E = 800000
P = 128
NCORES = 8
NSH = N // NCORES  # 6250
NQ = 9
QN = 5568  # nodes per src quarter (last quarter ragged: 5456)
QCOLS = 6272  # [zero, 5568 nodes, pad] = NPAD so tabs tiles serve finalize
NPAD = 6272  # padded node count (392*16) for perm gathers / acc width
MMCH = 512
PSHEAD = 4096  # acc cols merged in PSUM; tail NPAD-PSHEAD merged on DVE
NGROUP = 8
ZCH = 3136  # z-row DMA chunk cols (wide enough to dodge the <512B descriptor penalty)

BL1 = (10, 12, 14, 17, 21, 26, 32, 40, 48, 64, 96, 128, 192, 256, 384, 512)
BL2 = (10, 12, 14, 17, 21, 26, 32, 40, 48, 64)
EXACT1 = 8
EXACT2 = 8


def _pad(n, m):
    return ((n + m - 1) // m) * m


def _wrap16(a):
    n = a.shape[0]
    assert n % 16 == 0
    return np.ascontiguousarray(a.reshape(n // 16, 16).T)


def _make_lut(kmax, exact, bl):
    lut = np.arange(max(kmax, exact) + 1)
    for kk in range(exact + 1, kmax + 1):
        hit = [b for b in bl if kk <= b]
        lut[kk] = hit[0] if hit else kk
    return lut


def _pack_two_chunks(budget):
    """budget: {k: n_rows}. Pack ladder rows into <=2 chunks. Returns
    (cap, descr, ncols, kbase, used1): cap = chunk capacity in slots (%128),
    descr = [(ch, off, n_rows, k, col)], used1 = used slots in chunk 1."""
    slots = sum(k * n for k, n in budget.items())
    cap = _pad(max(slots // 2 + 64, QCOLS), 128)
    while True:
        descr, kbase = [], {}
        col, ch, off = 1, 0, 0
        ok = True
        for k in sorted(budget):
            nk = budget[k]
            kbase[k] = col
            left = nk
            while left > 0:
                fit = min(left, (cap - off) // k)
                if fit == 0:
                    ch += 1
                    off = 0
                    fit = min(left, cap // k)
                    if fit == 0 or ch > 1:
                        ok = False
                        break
                descr.append((ch, off, fit, k, col))
                off += fit * k
                col += fit
                left -= fit
            if not ok:
                break
        if ok:
            used1 = off if ch == 1 else 0
            return cap, descr, col, kbase, used1
        cap += 128


def _budgets(kapb):
    """kapb: [cells, nodes] bucketed degrees -> {k: max count over cells}"""
    b = {}
    for k in np.unique(kapb):
        k = int(k)
        if k == 0:
            continue
        nk = int((kapb == k).sum(axis=1).max())
        if nk > 0:
            b[k] = nk
    return b


def _fill_slots(kv, lut, descr, kbase, cap, d_edges, s_edges):
    """Per-cell slot assignment. kv: [nodes] actual degrees; d_edges/s_edges:
    edge dst-local indices / src-table values sorted by dst-local. Returns
    (eslots, evals, pm) with pm[node] = accp col (0 if no edges)."""
    kvb = lut[kv]
    nodes = np.nonzero(kv)[0]
    knb = kvb[nodes]
    nd = np.lexsort((nodes, knb))
    nodes_s, knb_s = nodes[nd], knb[nd]
    rank = np.zeros(len(nodes_s), dtype=np.int64)
    colof = np.zeros(len(nodes_s), dtype=np.int64)
    for k in np.unique(knb_s):
        mk = knb_s == k
        rank[mk] = np.arange(mk.sum())
        colof[mk] = kbase[int(k)]
    pm = np.zeros(kv.shape[0], dtype=np.int64)
    pm[nodes_s] = colof + rank
    ncol = max(d[4] + d[2] for d in descr)
    col2slot = np.full(ncol, -1, dtype=np.int64)
    for ch, off, n_rows, k, col in descr:
        cols = np.arange(n_rows)
        col2slot[col + cols] = ch * cap + off + cols * k
    ptr = np.zeros(kv.shape[0] + 1, dtype=np.int64)
    ptr[1:] = np.cumsum(kv)
    e_rank = np.arange(len(d_edges)) - ptr[d_edges]
    eslots = col2slot[pm[d_edges]] + e_rank
    return eslots, s_edges, pm


def host_prep_v2(x, edge_index, W1, b1, W2, b2):
    src = np.concatenate([edge_index[0], np.arange(N, dtype=np.int64)]).astype(np.int64)
    dst = np.concatenate([edge_index[1], np.arange(N, dtype=np.int64)]).astype(np.int64)
    deg = np.bincount(dst, minlength=N).astype(np.float32)
    dinv = (1.0 / np.sqrt(np.maximum(deg, 1e-12))).astype(np.float32)
    dinv[deg <= 0] = 0.0

    # src-table node permutation: spreads each node's self-loop into a
    # pseudo-random quarter, keeping per-(core,quarter) degree stats aligned
    psrc = np.random.default_rng(12345).permutation(N)
    pinv = np.argsort(psrc)

    xtp = (x * dinv[:, None]).T.astype(np.float32)[:, pinv]  # [128, N] pos order
    xt = np.zeros((P, NQ * QCOLS), dtype=np.float32)
    for q in range(NQ):
        w = min(QN, N - q * QN)
        xt[:, q * QCOLS + 1 : q * QCOLS + 1 + w] = xtp[:, q * QN : q * QN + w]

    core = dst // NSH
    dstl = dst % NSH
    pos = psrc[src]
    quarter = pos // QN
    srcl = (pos % QN).astype(np.int64) + 1

    # ---- layer 1 budgets / layouts ----
    kap = np.zeros((NCORES, NQ, NSH), dtype=np.int32)
    for c in range(NCORES):
        mc = core == c
        for q in range(NQ):
            m = mc & (quarter == q)
            kap[c, q] = np.bincount(dstl[m], minlength=NSH)
    lut1 = _make_lut(int(kap.max()), EXACT1, BL1)
    kapb = lut1[kap]
    layouts = [_pack_two_chunks(_budgets(kapb[:, q, :])) for q in range(NQ)]
    SQ = [2 * l[0] for l in layouts]
    PQ = _pad(max(l[2] for l in layouts), 16)

    eidx = np.zeros((NCORES, sum(SQ)), dtype=np.int16)
    perms = np.zeros((NCORES, NQ, NPAD), dtype=np.int16)
    order = np.lexsort((dstl, quarter, core))
    so, do_, qo, co = srcl[order], dstl[order], quarter[order], core[order]
    for c in range(NCORES):
        qbase = 0
        mc = co == c
        for q in range(NQ):
            m = mc & (qo == q)
            cap, descr, _, kbase, _ = layouts[q]
            eslots, evals, pm = _fill_slots(
                kap[c, q], lut1, descr, kbase, cap, do_[m], so[m]
            )
            eidx[c, qbase + eslots] = evals.astype(np.int16)
            perms[c, q, :NSH] = pm.astype(np.int16)
            qbase += SQ[q]

    eidx_w = np.zeros((NCORES, P, sum(SQ) // 16), dtype=np.int16)
    perm_w = np.zeros((NCORES, P, NQ * (NPAD // 16)), dtype=np.int16)
    for c in range(NCORES):
        eidx_w[c] = np.tile(_wrap16(eidx[c]), (NGROUP, 1))
        pw = np.concatenate([_wrap16(perms[c, q]) for q in range(NQ)], axis=1)
        perm_w[c] = np.tile(pw, (NGROUP, 1))

    dinvb = np.zeros((NCORES, P, NPAD), dtype=np.float32)
    for c in range(NCORES):
        dinvb[c, :, :NSH] = np.tile(dinv[c * NSH : (c + 1) * NSH], (P, 1))

    # ---- layer 2 (no self-loops; groups = src shards) ----
    src0 = edge_index[0].astype(np.int64)
    dst0 = edge_index[1].astype(np.int64)
    core2 = dst0 // NSH
    dstl2 = dst0 % NSH
    grp2 = src0 // NSH
    srcl2 = (src0 % NSH).astype(np.int64) + 1

    kap2 = np.zeros((NCORES, NGROUP, NSH), dtype=np.int32)
    for c in range(NCORES):
        mc = core2 == c
        for g in range(NGROUP):
            m = mc & (grp2 == g)
            kap2[c, g] = np.bincount(dstl2[m], minlength=NSH)
    lut2 = _make_lut(int(kap2.max()), EXACT2, BL2)
    layout2 = _pack_two_chunks(_budgets(lut2[kap2].reshape(NCORES * NGROUP, NSH)))
    SQ2 = 2 * layout2[0]
    P2 = _pad(layout2[2], 16)

    eidx2 = np.zeros((NCORES, NGROUP, SQ2), dtype=np.int16)
    perm2 = np.zeros((NCORES, NGROUP, NPAD), dtype=np.int16)
    order2 = np.lexsort((dstl2, grp2, core2))
    so2, do2, go2, co2 = srcl2[order2], dstl2[order2], grp2[order2], core2[order2]
    cap2, descr2, _, kbase2, _ = layout2
    for c in range(NCORES):
        mc = co2 == c
        for g in range(NGROUP):
            m = mc & (go2 == g)
            eslots, evals, pm = _fill_slots(
                kap2[c, g], lut2, descr2, kbase2, cap2, do2[m], so2[m]
            )
            eidx2[c, g, eslots] = evals.astype(np.int16)
            perm2[c, g, :NSH] = pm.astype(np.int16)

    eidx2_w = np.zeros((NCORES, P, SQ2 // 16), dtype=np.int16)
    perm2_w = np.zeros((NCORES, P, NPAD // 16), dtype=np.int16)
    for c in range(NCORES):
        for g in range(NGROUP):
            eidx2_w[c, g * 16 : (g + 1) * 16] = _wrap16(eidx2[c, g])
            perm2_w[c, g * 16 : (g + 1) * 16] = _wrap16(perm2[c, g])

    meta = dict(
        layouts=layouts, SQ=SQ, PQ=PQ, layout2=layout2, SQ2=SQ2, P2=P2,
        b2=float(np.asarray(b2).reshape(-1)[0]),
    )
    ident = np.eye(P, dtype=np.float32)
    ones = np.ones((P, 1), dtype=np.float32)
    in_maps = []
    for c in range(NCORES):
        in_maps.append(
            {
                "xt": xt,
                "w1": np.asarray(W1, dtype=np.float32).astype(ml_dtypes.bfloat16),
                "ident": ident,
                "ones": ones,
                "b1": np.asarray(b1, dtype=np.float32).reshape(P, 1),
                "w2": np.asarray(W2, dtype=np.float32).reshape(P, 1).astype(ml_dtypes.bfloat16),
                "eidx": np.ascontiguousarray(eidx_w[c]),
                "perm": np.ascontiguousarray(perm_w[c]),
                "dinvb": np.ascontiguousarray(dinvb[c]),
                "eidx2": np.ascontiguousarray(eidx2_w[c]),
                "perm2": np.ascontiguousarray(perm2_w[c]),
            }
        )
    return in_maps, meta


def build_fused_v2(meta):
    layouts, SQ, PQ = meta["layouts"], meta["SQ"], meta["PQ"]
    layout2, SQ2, P2 = meta["layout2"], meta["SQ2"], meta["P2"]
    b2 = meta["b2"]
    PQA = _pad(max(PQ, P2), 16)  # accpA also serves layer 2
    PQB = _pad(PQ, 16)
    G1MAX = max(max(l[0] for l in layouts), layout2[0])
    TAILW = NPAD - PSHEAD
    EW = max(2 * max(l[0] for l in layouts) // 16, SQ2 // 16)

    nc = bacc.Bacc(None, target_bir_lowering=False)
    f32, bf16, i16 = mybir.dt.float32, mybir.dt.bfloat16, mybir.dt.int16
    AF = mybir.ActivationFunctionType
    xt_d = nc.dram_tensor("xt", [P, NQ * QCOLS], f32, kind="ExternalInput")
    w1_d = nc.dram_tensor("w1", [P, P], bf16, kind="ExternalInput")
    ident_d = nc.dram_tensor("ident", [P, P], f32, kind="ExternalInput")
    ones_d = nc.dram_tensor("ones", [P, 1], f32, kind="ExternalInput")
    b1_d = nc.dram_tensor("b1", [P, 1], f32, kind="ExternalInput")
    w2_d = nc.dram_tensor("w2", [P, 1], bf16, kind="ExternalInput")
    eidx_d = nc.dram_tensor("eidx", [P, sum(SQ) // 16], i16, kind="ExternalInput")
    perm_d = nc.dram_tensor("perm", [P, NQ * (NPAD // 16)], i16, kind="ExternalInput")
    dinvb_d = nc.dram_tensor("dinvb", [P, NPAD], f32, kind="ExternalInput")
    eidx2_d = nc.dram_tensor("eidx2", [P, SQ2 // 16], i16, kind="ExternalInput")
    perm2_d = nc.dram_tensor("perm2", [P, NPAD // 16], i16, kind="ExternalInput")
    out_d = nc.dram_tensor("out2", [1, NSH], f32, kind="ExternalOutput")
    zin = nc.dram_tensor("zin_cc", [NPAD // ZCH, ZCH], f32, kind="Internal")
    zall = nc.dram_tensor("zall_cc", [NCORES * (NPAD // ZCH), ZCH], f32, kind="Internal", addr_space="Shared")

    with ExitStack() as ctx:
        tc = ctx.enter_context(TileContext(nc))
        cpool = ctx.enter_context(tc.tile_pool(name="cpool", bufs=1))
        w1 = cpool.tile([P, P], bf16)
        ident = cpool.tile([P, P], f32)
        ones = cpool.tile([P, 1], f32)
        b1 = cpool.tile([P, 1], f32)
        w2 = cpool.tile([P, 1], bf16)
        acct = cpool.tile([P, TAILW], bf16)
        accpA = cpool.tile([P, PQA], f32)
        PQW = [PQA, PQB]
        dinvb = cpool.tile([P, NPAD], f32)
        nc.sync.dma_start(out=w1[:], in_=w1_d[:])
        nc.sync.dma_start(out=ident[:], in_=ident_d[:])
        nc.sync.dma_start(out=ones[:], in_=ones_d[:])
        nc.sync.dma_start(out=b1[:], in_=b1_d[:])
        nc.sync.dma_start(out=w2[:], in_=w2_d[:])
        nc.vector.memset(accpA[:, 0:1], 0.0)

        gpool = ctx.enter_context(tc.tile_pool(name="gpool", bufs=2))
        epool = ctx.enter_context(tc.tile_pool(name="epool", bufs=2))
        ppool = ctx.enter_context(tc.tile_pool(name="ppool", bufs=2))

        def emit_merge(q, t):
            """perm-gather quarter q's accp into node order."""
            nc.gpsimd.ap_gather(
                t[:], accps[q % 2][:], psls[q][:],
                channels=P, num_elems=PQW[q % 2], d=1, num_idxs=NPAD,
            )

        def emit_merge_consume(q, t):
            for i in range(PSHEAD // MMCH):
                c0 = i * MMCH
                nc.tensor.matmul(
                    psAs[i][:], ident[:], t[:, c0 : c0 + MMCH],
                    start=(q == 0), stop=(q == NQ - 1),
                )
            if q == 0:
                nc.vector.tensor_copy(acct[:], t[:, PSHEAD:])
            else:
                nc.vector.tensor_add(acct[:], acct[:], t[:, PSHEAD:])

        psls = {}
        tabs = ctx.enter_context(tc.tile_pool(name="tabs", bufs=2))
        tpool = ctx.enter_context(tc.tile_pool(name="tpool", bufs=1))
        bpool = ctx.enter_context(tc.tile_pool(name="bpool", bufs=1))
        with tc.tile_pool(name="psmerge", bufs=1, space="PSUM") as psmerge:
            psAs = [
                psmerge.tile([P, MMCH], f32, name=f"psA{i}")
                for i in range(PSHEAD // MMCH)
            ]
            if True:
                accpB = bpool.tile([P, PQB], f32)
                accps = [accpA, accpB]
                nc.vector.memset(accpB[:, 0:1], 0.0)
                ebase = 0
                prev_t = None
                prev_q = -1
                for q in range(NQ):
                    cap, descr, _, _, used1 = layouts[q]
                    tab = tabs.tile([P, QCOLS], f32, tag="tab")
                    esl = epool.tile([P, EW], i16, tag="e")
                    psl = ppool.tile([P, NPAD // 16], i16, tag="p")
                    psls[q] = psl
                    nc.sync.dma_start(
                        out=esl[:, : SQ[q] // 16],
                        in_=eidx_d[:, ebase // 16 : (ebase + SQ[q]) // 16],
                    )
                    nc.sync.dma_start(
                        out=psl[:],
                        in_=perm_d[:, q * (NPAD // 16) : (q + 1) * (NPAD // 16)],
                    )
                    if q == 0:
                        qrt = QCOLS // 4
                        for ji, eng in enumerate((nc.sync, nc.scalar, nc.gpsimd, nc.gpsimd)):
                            c0 = ji * qrt
                            eng.dma_start(
                                out=tab[:, c0 : c0 + qrt],
                                in_=xt_d[:, q * QCOLS + c0 : q * QCOLS + c0 + qrt],
                            )
                    else:
                        half = _pad(QCOLS // 2, 16)
                        nc.sync.dma_start(
                            out=tab[:, :half], in_=xt_d[:, q * QCOLS : q * QCOLS + half]
                        )
                        nc.scalar.dma_start(
                            out=tab[:, half:], in_=xt_d[:, q * QCOLS + half : (q + 1) * QCOLS]
                        )
                    if q == 1:
                        # preload the sigmoid act table off the critical path
                        nc.scalar.activation(acct[0:1, 0:1], b1[0:1, 0:1], AF.Sigmoid)
                    if q == 5:
                        nc.sync.dma_start(out=dinvb[:], in_=dinvb_d[:])
                    by_chunk = {0: [], 1: []}
                    for d_ in descr:
                        by_chunk[d_[0]].append(d_)
                    gs = []
                    for ch in range(2):
                        if not by_chunk[ch]:
                            continue
                        ni = cap if ch == 0 else max(_pad(used1, 16), 16)
                        g = gpool.tile([P, G1MAX], f32, tag="g")
                        gs.append((g, ni, by_chunk[ch], ch))
                    # Pool order: gatherA_q, perm_{q-1}, gatherB_q -> no
                    # stalls (accp double-buffered so quarter q's reduces
                    # don't WAR against perm_{q-1})
                    g, ni, rows, _ = gs[0]
                    nc.gpsimd.ap_gather(
                        g[:, :ni], tab[:], esl[:, : ni // 16],
                        channels=P, num_elems=QCOLS, d=1, num_idxs=ni,
                    )
                    if len(gs) > 1:
                        g2_, ni2, rows2, _ = gs[1]
                        nc.gpsimd.ap_gather(
                            g2_[:, :ni2], tab[:],
                            esl[:, cap // 16 : cap // 16 + ni2 // 16],
                            channels=P, num_elems=QCOLS, d=1, num_idxs=ni2,
                        )
                    if prev_t is not None:
                        emit_merge(prev_q, prev_t)
                    for (_, off, n_rows, k, col) in rows:
                        nc.vector.tensor_reduce(
                            accps[q % 2][:, col : col + n_rows],
                            g[:, off : off + n_rows * k].rearrange(
                                "p (a b) -> p a b", a=n_rows, b=k
                            ),
                            axis=mybir.AxisListType.X, op=mybir.AluOpType.add,
                        )
                    if prev_t is not None:
                        emit_merge_consume(prev_q, prev_t)
                    if len(gs) > 1:
                        for (_, off, n_rows, k, col) in rows2:
                            nc.vector.tensor_reduce(
                                accps[q % 2][:, col : col + n_rows],
                                g2_[:, off : off + n_rows * k].rearrange(
                                    "p (a b) -> p a b", a=n_rows, b=k
                                ),
                                axis=mybir.AxisListType.X, op=mybir.AluOpType.add,
                            )
                    prev_t = tpool.tile([P, NPAD], f32, tag="t")
                    prev_q = q
                    ebase += SQ[q]
                emit_merge(prev_q, prev_t)
                emit_merge_consume(prev_q, prev_t)

            # psA banks are separate tiles, so each drain mul chases its own
            # bank's final matmul (pipelined PE->DVE, no pool barrier)
            acc_t = tabs.tile([P, QCOLS], f32, tag="tab")
            acc = acc_t[:].bitcast(bf16)  # [P, 2*QCOLS] bf16 view; use :NPAD
            for i in range(PSHEAD // MMCH):
                c0 = i * MMCH
                nc.vector.tensor_mul(
                    acc[:, c0 : c0 + MMCH], psAs[i][:], dinvb[:, c0 : c0 + MMCH]
                )
            nc.vector.tensor_mul(acc[:, PSHEAD:NPAD], acct[:], dinvb[:, PSHEAD:])

        # ---- finalize layer 1: h = sigmoid(W1^T acc + b1); z = dinv * W2^T h
        zt = tpool.tile([P, NPAD], f32, tag="t")
        nc.gpsimd.memset(zt[:], 0.0)
        with (
            tc.tile_pool(name="psh", bufs=2, space="PSUM") as psh,
            tc.tile_pool(name="psz", bufs=2, space="PSUM") as psz,
        ):
            h_t = tabs.tile([P, QCOLS], f32, tag="tab")
            h = h_t[:].bitcast(bf16)
            for c0 in range(0, NPAD, MMCH):
                w = min(MMCH, NPAD - c0)
                ps = psh.tile([P, MMCH], f32, tag="h")
                nc.tensor.matmul(ps[:, :w], w1[:], acc[:, c0 : c0 + w], start=True, stop=True)
                nc.scalar.activation(h[:, c0 : c0 + w], ps[:, :w], AF.Sigmoid, bias=b1[:, 0:1])
            zrow = tabs.tile([P, QCOLS], f32, tag="tab")
            nc.scalar.memzero(zrow[0:1, NSH:])
            for c0 in range(0, NSH, MMCH):
                w = min(MMCH, NSH - c0)
                ps = psz.tile([1, MMCH], f32, tag="z")
                nc.tensor.matmul(ps[:, :w], w2[:], h[:, c0 : c0 + w], start=True, stop=True)
                nc.vector.tensor_mul(
                    zrow[0:1, c0 : c0 + w], ps[0:1, :w], dinvb[0:1, c0 : c0 + w]
                )
            zrv = zrow[0:1, :NPAD].rearrange("p (a b) -> p a b", a=NPAD // ZCH, b=ZCH)
            for j in range(4):
                c0 = j * (ZCH // 4)
                eng = nc.sync if j % 2 == 0 else nc.scalar
                eng.dma_start(
                    out=zin[:, c0 : c0 + ZCH // 4], in_=zrv[:, :, c0 : c0 + ZCH // 4]
                )
        # prefetch layer-2 index data while the collective runs
        esl2 = epool.tile([P, EW], i16, tag="e")
        nc.scalar.dma_start(out=esl2[:, : SQ2 // 16], in_=eidx2_d[:])
        psl2 = ppool.tile([P, NPAD // 16], i16, tag="p")
        nc.scalar.dma_start(out=psl2[:], in_=perm2_d[:])

        nc.gpsimd.collective_compute(
            "AllGather", mybir.AluOpType.bypass,
            replica_groups=[list(range(NCORES))],
            ins=[zin[:].opt()], outs=[zall[:].opt()],
        )

        # ---- layer 2 ----
        cap2, descr2, _, _, used1b = layout2
        # z table on stripe partitions 16g; column-chunked across the SP and
        # Act DMA queues (the cost model charges per-partition-line bytes)
        NB = NPAD // ZCH
        for j in range(NB):
            c0 = j * ZCH
            w = min(ZCH, NSH - c0)
            eng = nc.sync if j % 2 == 0 else nc.scalar
            eng.dma_start(
                out=zt[0 : NGROUP * 16 : 16, 1 + c0 : 1 + c0 + w],
                in_=zall[j : NCORES * NB : NB, :w],
            )
        by_chunk2 = {0: [], 1: []}
        for d_ in descr2:
            by_chunk2[d_[0]].append(d_)
        for ch in range(2):
            if not by_chunk2[ch]:
                continue
            ni = cap2 if ch == 0 else max(_pad(used1b, 16), 16)
            g = gpool.tile([P, G1MAX], f32, tag="g")
            nc.gpsimd.ap_gather(
                g[:, :ni], zt[:],
                esl2[:, ch * cap2 // 16 : ch * cap2 // 16 + ni // 16],
                channels=P, num_elems=NPAD, d=1, num_idxs=ni,
            )
            rows2 = by_chunk2[ch]
            pool_elems = 0
            budget = 0  # gpsimd tensor_reduce only supports axis C
            for (_, off, n_rows, k, col) in rows2:
                eng = nc.vector
                if ch == 1 and pool_elems + n_rows * k <= budget:
                    eng = nc.gpsimd
                    pool_elems += n_rows * k
                eng.tensor_reduce(
                    accpA[:, col : col + n_rows],
                    g[:, off : off + n_rows * k].rearrange("p (a b) -> p a b", a=n_rows, b=k),
                    axis=mybir.AxisListType.X, op=mybir.AluOpType.add,
                )
        t2 = tabs.tile([P, QCOLS], f32, tag="tab")  # evicts h
        nc.gpsimd.ap_gather(
            t2[:], accpA[:], psl2[:],
            channels=P, num_elems=PQA, d=1, num_idxs=NPAD,
        )
        # self-loop z' injected as diagonal blocks on spare partitions 1..8
        # (masked partition-sum ignores all other non-stripe rows), DMAs
        # interleaved with the partition-sum matmuls that consume them
        orow_t = tabs.tile([P, QCOLS], f32, tag="tab")  # evicts zrow
        HB = ZCH // 2  # 1568-col block pieces (row-slices stay >=512B/descr)
        with tc.tile_pool(name="pso", bufs=2, space="PSUM") as pso:
            nextblk = 0
            for c0 in range(0, NSH, MMCH):
                w = min(MMCH, NSH - c0)
                need = (c0 + w - 1) // HB
                while nextblk <= min(need, NPAD // HB - 1):
                    p = nextblk
                    j, k = p // 2, p % 2
                    b0 = p * HB
                    bw = min(HB, NSH - b0)
                    eng = nc.sync if p % 2 == 0 else nc.scalar
                    eng.dma_start(
                        out=t2[1 + j : 2 + j, b0 : b0 + bw],
                        in_=zin[j : j + 1, k * HB : k * HB + bw],
                    )
                    nextblk += 1
                ps = pso.tile([1, MMCH], f32, tag="o")
                nc.tensor.matmul(ps[:, :w], ones[:], t2[:, c0 : c0 + w], start=True, stop=True)
                nc.vector.tensor_mul(
                    orow_t[0:1, c0 : c0 + w], ps[0:1, :w], dinvb[0:1, c0 : c0 + w]
                )
                nc.scalar.activation(
                    orow_t[0:1, c0 : c0 + w], orow_t[0:1, c0 : c0 + w], AF.Sigmoid, bias=b2
                )
            nc.sync.dma_start(out=out_d[:], in_=orow_t[0:1, :NSH])
    nc.finalize()
    return nc


def _sim_ns(nc):
    from concourse import bass_interp

    sim = bass_interp.CoreSim(nc, no_exec=True, publish_trace=False)
    sim.simulate()
    return int(sim.time)


def kernel_v2(x, edge_index, W1, b1, W2, b2):
    global LAST_SIM_NS
    x = np.asarray(x, dtype=np.float32)
    edge_index = np.asarray(edge_index)
    in_maps, meta = host_prep_v2(x, edge_index, W1, b1, W2, b2)
    nc = build_fused_v2(meta)
    if MEASURE:
        LAST_SIM_NS = _sim_ns(nc)
    res = run_bass_kernel_spmd(nc, in_maps, list(range(NCORES)))
    out = np.zeros((N, 1), dtype=np.float32)
    for c in range(NCORES):
        out[c * NSH : (c + 1) * NSH, 0] = res.results[c]["out2"][0, :NSH]
    return out


kernel = kernel_v2
